# revision 1
# baseline (speedup 1.0000x reference)
"""MoE gate (top-6 routing) Trainium2 Bass kernel.

Problem: hidden_states [4, 4096, 2048] f32, gate weight [64, 2048] f32.
  logits = x @ W.T            -> [16384, 64]
  topk_weight, topk_idx = top_k(logits, 6)
  topk_weight = softmax(topk_weight)   (the reference's extra
  normalization divides by 1.0 + 1e-20 and is a no-op in fp32)
Returns (topk_idx int32 [16384, 6], topk_weight f32 [16384, 6]).

Sharding: data-parallel over tokens. Each of the 8 cores gets 2048
tokens; the gate weight is replicated. The token shard is fed to the
device pre-transposed ([H, T] layout) so the contraction dim lands on
SBUF partitions with fully contiguous DMA loads.

Per-core kernel (all fp32 — bf16/tf32 would flip near-tied expert
rankings vs the fp32 reference):
  - load W.T into SBUF as 16 k-tiles [128, 64]
  - load x.T as 16 k-tiles [128, 2048] (1 MiB DMAs)
  - per 128-token tile: 16 accumulated matmuls (lhsT = x-tile slice,
    stationary; rhs = w-tile, moving) -> PSUM [128 tok, 64 expert]
  - DVE max8/max_index -> top-8 values + indices per token
  - ACT exp(v - max) with accumulated sum, DVE reciprocal + scale
  - results staged in SBUF, two DMAs out per core; host de-interleaves
"""

import numpy as np

import concourse.bass as bass
import concourse.mybir as mybir
import concourse.tile as tile
from concourse import bacc
from concourse.bass_utils import run_bass_kernel_spmd

f32 = mybir.dt.float32
u32 = mybir.dt.uint32
i32 = mybir.dt.int32

N_CORES = 8
B, S, H = 4, 4096, 2048
E = 64
TOP_K = 6
T_FULL = B * S            # 16384 tokens
T_CORE = T_FULL // N_CORES  # 2048 tokens per core
KT = H // 128             # 16 contraction tiles
NTT = T_CORE // 128       # 16 token tiles per core

_CACHE = {}


def _build():
    nc = bacc.Bacc("TRN2", target_bir_lowering=False, debug=False)
    xT = nc.dram_tensor("xT", [H, T_CORE], f32, kind="ExternalInput").ap()
    wT = nc.dram_tensor("wT", [H, E], f32, kind="ExternalInput").ap()
    out_w = nc.dram_tensor("out_w", [128, NTT * TOP_K], f32, kind="ExternalOutput").ap()
    out_i = nc.dram_tensor("out_i", [128, NTT * TOP_K], i32, kind="ExternalOutput").ap()

    with tile.TileContext(nc) as tc:
        with (
            tc.tile_pool(name="persist", bufs=1) as persist,
            tc.tile_pool(name="work", bufs=4) as work,
            tc.tile_pool(name="psum", bufs=4, space="PSUM") as psp,
            tc.tile_pool(name="psumW", bufs=1, space="PSUM") as pspW,
        ):
            w_all = persist.tile([128, KT * E], f32, tag="w_all")
            nc.sync.dma_start(
                out=w_all.rearrange("p (a e) -> p a e", a=KT),
                in_=wT.rearrange("(a p) e -> p a e", p=128),
            )

            x_tiles = []
            for a in range(KT):
                xt = persist.tile([128, T_CORE], f32, tag=f"x{a}")
                nc.sync.dma_start(out=xt, in_=xT[a * 128 : (a + 1) * 128, :])
                x_tiles.append(xt)

            # Warmup matmul: absorbs the w-DMA wait on the PE so every real
            # (fused fp32) matmul needs at most one semaphore wait.
            ps_warm = pspW.tile([64, 64], f32, tag="ps_warm")
            nc.tensor.matmul(
                ps_warm, w_all[:, 0:64], w_all[:, 0:64], start=True, stop=True
            )

            stage_w = persist.tile([128, NTT * TOP_K], f32, tag="stage_w")
            stage_i = persist.tile([128, NTT * TOP_K], u32, tag="stage_i")

            for t in range(NTT):
                ps = psp.tile([128, E], f32, tag="ps")
                for a in range(KT):
                    nc.tensor.matmul(
                        ps,
                        x_tiles[a][:, t * 128 : (t + 1) * 128],
                        w_all[:, a * E : (a + 1) * E],
                        start=(a == 0),
                        stop=(a == KT - 1),
                    )
                la = work.tile([128, E], f32, tag="la")
                nc.scalar.copy(out=la, in_=ps)

                m8 = work.tile([128, 8], f32, tag="m8")
                nc.vector.max(out=m8, in_=la)
                i8 = work.tile([128, 8], u32, tag="i8")
                nc.vector.max_index(i8, m8, la)
                nc.vector.tensor_copy(
                    stage_i[:, t * TOP_K : (t + 1) * TOP_K], i8[:, :TOP_K]
                )

                negm = work.tile([128, 1], f32, tag="negm")
                nc.scalar.mul(negm, m8[:, 0:1], -1.0)
                expw = work.tile([128, TOP_K], f32, tag="expw")
                ssum = work.tile([128, 1], f32, tag="ssum")
                nc.scalar.activation(
                    out=expw,
                    in_=m8[:, 0:TOP_K],
                    func=mybir.ActivationFunctionType.Exp,
                    bias=negm[:, 0:1],
                    scale=1.0,
                    accum_out=ssum[:, 0:1],
                )
                rsum = work.tile([128, 1], f32, tag="rsum")
                nc.vector.reciprocal(rsum, ssum)
                nc.vector.tensor_scalar_mul(
                    stage_w[:, t * TOP_K : (t + 1) * TOP_K], expw, rsum[:, 0:1]
                )

            nc.gpsimd.dma_start(out=out_w, in_=stage_w)
            nc.gpsimd.dma_start(out=out_i, in_=stage_i.bitcast(i32))

    nc.compile()
    return nc


def _get_nc():
    if "nc" not in _CACHE:
        _CACHE["nc"] = _build()
    return _CACHE["nc"]


def kernel(hidden_states: np.ndarray, weight: np.ndarray, **_run_kwargs):
    x = np.ascontiguousarray(hidden_states, dtype=np.float32).reshape(T_FULL, H)
    w = np.ascontiguousarray(weight, dtype=np.float32)

    wT = np.ascontiguousarray(w.T)  # [H, E]
    in_maps = []
    for c in range(N_CORES):
        shard = x[c * T_CORE : (c + 1) * T_CORE, :]  # [T_CORE, H]
        xT = np.ascontiguousarray(shard.T)  # [H, T_CORE]
        in_maps.append({"xT": xT, "wT": wT})

    nc = _get_nc()
    res = run_bass_kernel_spmd(
        nc, in_maps, core_ids=list(range(N_CORES)), **_run_kwargs
    )

    idx_parts = []
    w_parts = []
    for c in range(N_CORES):
        r = res.results[c]
        # staged [128, NTT*K]: row p, col t*K+k  ->  token t*128+p, slot k
        si = r["out_i"].reshape(128, NTT, TOP_K).transpose(1, 0, 2).reshape(T_CORE, TOP_K)
        sw = r["out_w"].reshape(128, NTT, TOP_K).transpose(1, 0, 2).reshape(T_CORE, TOP_K)
        idx_parts.append(si.astype(np.int32, copy=False))
        w_parts.append(sw)

    topk_idx = np.concatenate(idx_parts, axis=0)
    topk_weight = np.concatenate(w_parts, axis=0)
    if "trace" in _run_kwargs:
        return (topk_idx, topk_weight), res
    return topk_idx, topk_weight


# revision 3
# speedup vs baseline: 1.9161x; 1.9161x over previous
"""MoE gate (top-6 routing) Trainium2 Bass kernel.

Problem: hidden_states [4, 4096, 2048] f32, gate weight [64, 2048] f32.
  logits = x @ W.T            -> [16384, 64]
  topk_weight, topk_idx = top_k(logits, 6)
  topk_weight = softmax(topk_weight)   (the reference's extra
  normalization divides by 1.0 + 1e-20 and is a no-op in fp32)
Returns (topk_idx int32 [16384, 6], topk_weight f32 [16384, 6]).

Sharding: data-parallel over tokens. Each of the 8 cores gets 2048
tokens; the gate weight is replicated. The token shard is fed to the
device pre-transposed ([H, T] layout) so the contraction dim lands on
SBUF partitions with fully contiguous DMA loads.

Per-core kernel (all fp32 — bf16/tf32 would flip near-tied expert
rankings vs the fp32 reference):
  - load W.T into SBUF as 16 k-tiles [128, 64]
  - load x.T as 16 k-tiles [128, 2048] (1 MiB DMAs)
  - matmuls in [E, T'] orientation (w-tile stationary, x moving at
    N=512), 2-way column-tiled: two token-blocks accumulate
    concurrently into partition halves [0:64] / [64:128] of one PSUM
    bank (distinct col-groups of the PE array -> concurrent streams)
  - PE-transpose of the small logits to [token, expert] tiles
  - DVE max8/max_index -> top-8 values + indices per token
  - ACT exp(v - max) with accumulated sum, DVE reciprocal + scale
  - results staged in SBUF, two DMAs out per core; host de-interleaves
"""

import numpy as np

import concourse.bass as bass
import concourse.mybir as mybir
import concourse.tile as tile
from concourse import bacc
from concourse.bass_utils import run_bass_kernel_spmd

f32 = mybir.dt.float32
u32 = mybir.dt.uint32
i32 = mybir.dt.int32

N_CORES = 8
B, S, H = 4, 4096, 2048
E = 64
TOP_K = 6
T_FULL = B * S            # 16384 tokens
T_CORE = T_FULL // N_CORES  # 2048 tokens per core
KT = H // 128             # 16 contraction tiles
NTT = T_CORE // 128       # 16 token tiles per core
TB = 512                  # tokens per matmul block (PSUM bank = 512 fp32)
NB = T_CORE // TB         # 4 blocks per core -> 2 packed psum tiles

_CACHE = {}


def _build():
    nc = bacc.Bacc("TRN2", target_bir_lowering=False, debug=False)
    xT = nc.dram_tensor("xT", [H, T_CORE], f32, kind="ExternalInput").ap()
    wT = nc.dram_tensor("wT", [H, E], f32, kind="ExternalInput").ap()
    ident = nc.dram_tensor("ident", [E, E], f32, kind="ExternalInput").ap()
    out_w = nc.dram_tensor("out_w", [128, NTT * TOP_K], f32, kind="ExternalOutput").ap()
    out_i = nc.dram_tensor("out_i", [128, NTT * TOP_K], i32, kind="ExternalOutput").ap()

    with tile.TileContext(nc) as tc:
        with (
            tc.tile_pool(name="persist", bufs=1) as persist,
            tc.tile_pool(name="work", bufs=4) as work,
            tc.tile_pool(name="psum", bufs=1, space="PSUM") as psp,
            tc.tile_pool(name="psumT", bufs=4, space="PSUM") as pspT,
            tc.tile_pool(name="psumW", bufs=1, space="PSUM") as pspW,
        ):
            w_all = persist.tile([128, KT * E], f32, tag="w_all")
            nc.sync.dma_start(
                out=w_all.rearrange("p (a e) -> p a e", a=KT),
                in_=wT.rearrange("(a p) e -> p a e", p=128),
            )
            id_t = persist.tile([E, E], f32, tag="ident")
            nc.sync.dma_start(out=id_t, in_=ident)

            x_tiles = []
            for a in range(KT):
                xt = persist.tile([128, T_CORE], f32, tag=f"x{a}")
                nc.sync.dma_start(out=xt, in_=xT[a * 128 : (a + 1) * 128, :])
                x_tiles.append(xt)

            # Warmup matmuls: absorb the w/ident DMA waits on the PE so every
            # real (fused fp32) matmul carries at most one semaphore wait.
            ps_warm = pspW.tile([64, 64], f32, tag="ps_warm")
            nc.tensor.matmul(ps_warm, id_t, id_t, start=True, stop=True)
            nc.tensor.matmul(
                ps_warm, w_all[:, 0:64], w_all[:, 0:64], start=True, stop=True
            )

            stage_w = persist.tile([128, NTT * TOP_K], f32, tag="stage_w")
            stage_i = persist.tile([128, NTT * TOP_K], u32, tag="stage_i")

            # logits.T accumulation: psum tile p holds blocks 2p (partitions
            # 0:64) and 2p+1 (partitions 64:128), each [64 experts, 512 toks]
            ps_tiles = []
            for p in range(NB // 2):
                ps = psp.tile([128, TB], f32, tag=f"ps{p}")
                ps_tiles.append(ps)
            for a in range(KT):
                w_tile = w_all[:, a * E : (a + 1) * E]
                for p in range(NB // 2):
                    for half in range(2):
                        b = 2 * p + half
                        nc.tensor.matmul(
                            ps_tiles[p][half * 64 : (half + 1) * 64, :],
                            w_tile,
                            x_tiles[a][:, b * TB : (b + 1) * TB],
                            start=(a == 0),
                            stop=(a == KT - 1),
                        )

            # copy each block's logits.T to SBUF (base partition 0 for the
            # PE transposes), then transpose to [128 tok, 64 expert] tiles
            # and run the top-k + softmax chain.
            for p in range(NB // 2):
                for half in range(2):
                    b = 2 * p + half
                    ltE = work.tile([64, TB], f32, tag="ltE")
                    nc.scalar.copy(
                        out=ltE, in_=ps_tiles[p][half * 64 : (half + 1) * 64, :]
                    )
                    for tt in range(TB // 128):
                        t = b * (TB // 128) + tt  # global token tile in core
                        ps_t = pspT.tile([128, E], f32, tag="ps_t")
                        nc.tensor.transpose(
                            ps_t, ltE[:, tt * 128 : (tt + 1) * 128], id_t
                        )
                        la = work.tile([128, E], f32, tag="la")
                        nc.scalar.copy(out=la, in_=ps_t)

                        m8 = work.tile([128, 8], f32, tag="m8")
                        nc.vector.max(out=m8, in_=la)
                        i8 = work.tile([128, 8], u32, tag="i8")
                        nc.vector.max_index(i8, m8, la)
                        nc.vector.tensor_copy(
                            stage_i[:, t * TOP_K : (t + 1) * TOP_K], i8[:, :TOP_K]
                        )

                        negm = work.tile([128, 1], f32, tag="negm")
                        nc.scalar.mul(negm, m8[:, 0:1], -1.0)
                        expw = work.tile([128, TOP_K], f32, tag="expw")
                        ssum = work.tile([128, 1], f32, tag="ssum")
                        nc.scalar.activation(
                            out=expw,
                            in_=m8[:, 0:TOP_K],
                            func=mybir.ActivationFunctionType.Exp,
                            bias=negm[:, 0:1],
                            scale=1.0,
                            accum_out=ssum[:, 0:1],
                        )
                        rsum = work.tile([128, 1], f32, tag="rsum")
                        nc.vector.reciprocal(rsum, ssum)
                        nc.vector.tensor_scalar_mul(
                            stage_w[:, t * TOP_K : (t + 1) * TOP_K],
                            expw,
                            rsum[:, 0:1],
                        )

            nc.gpsimd.dma_start(out=out_w, in_=stage_w)
            nc.gpsimd.dma_start(out=out_i, in_=stage_i.bitcast(i32))

    nc.compile()
    return nc


def _get_nc():
    if "nc" not in _CACHE:
        _CACHE["nc"] = _build()
    return _CACHE["nc"]


def kernel(hidden_states: np.ndarray, weight: np.ndarray, **_run_kwargs):
    x = np.ascontiguousarray(hidden_states, dtype=np.float32).reshape(T_FULL, H)
    w = np.ascontiguousarray(weight, dtype=np.float32)

    wT = np.ascontiguousarray(w.T)  # [H, E]
    ident = np.eye(E, dtype=np.float32)
    in_maps = []
    for c in range(N_CORES):
        shard = x[c * T_CORE : (c + 1) * T_CORE, :]  # [T_CORE, H]
        xT = np.ascontiguousarray(shard.T)  # [H, T_CORE]
        in_maps.append({"xT": xT, "wT": wT, "ident": ident})

    nc = _get_nc()
    res = run_bass_kernel_spmd(
        nc, in_maps, core_ids=list(range(N_CORES)), **_run_kwargs
    )

    idx_parts = []
    w_parts = []
    for c in range(N_CORES):
        r = res.results[c]
        # staged [128, NTT*K]: row p, col t*K+k  ->  token t*128+p, slot k
        si = r["out_i"].reshape(128, NTT, TOP_K).transpose(1, 0, 2).reshape(T_CORE, TOP_K)
        sw = r["out_w"].reshape(128, NTT, TOP_K).transpose(1, 0, 2).reshape(T_CORE, TOP_K)
        idx_parts.append(si.astype(np.int32, copy=False))
        w_parts.append(sw)

    topk_idx = np.concatenate(idx_parts, axis=0)
    topk_weight = np.concatenate(w_parts, axis=0)
    if "trace" in _run_kwargs:
        return (topk_idx, topk_weight), res
    return topk_idx, topk_weight


# revision 4
# speedup vs baseline: 2.0025x; 1.0451x over previous
"""MoE gate (top-6 routing) Trainium2 Bass kernel.

Problem: hidden_states [4, 4096, 2048] f32, gate weight [64, 2048] f32.
  logits = x @ W.T            -> [16384, 64]
  topk_weight, topk_idx = top_k(logits, 6)
  topk_weight = softmax(topk_weight)   (the reference's extra
  normalization divides by 1.0 + 1e-20 and is a no-op in fp32)
Returns (topk_idx int32 [16384, 6], topk_weight f32 [16384, 6]).

Sharding: data-parallel over tokens. Each of the 8 cores gets 2048
tokens; the gate weight is replicated. The token shard is fed to the
device pre-transposed ([H, T] layout) so the contraction dim lands on
SBUF partitions with fully contiguous DMA loads.

Per-core kernel (all fp32 — bf16/tf32 would flip near-tied expert
rankings vs the fp32 reference):
  - two 1024-token super-panels, streamed panel-major so panel 0's
    top-k overlaps panel 1's DMA
  - matmuls in [E, T'] orientation (w-tile stationary, x moving at
    N=512), 2-way column-tiled: a panel's two 512-token blocks
    accumulate concurrently into partition halves [0:64] / [64:128]
    of one PSUM bank (distinct col-groups -> concurrent streams)
  - PE-transpose of the small logits to [token, expert] tiles
  - DVE max8/max_index (straight from PSUM) -> top-8 + indices
  - ACT exp(v - max) with accumulated sum, DVE reciprocal + scale
  - results staged in SBUF, per-panel DMAs out; host de-interleaves
"""

import numpy as np

import concourse.bass as bass
import concourse.mybir as mybir
import concourse.tile as tile
from concourse import bacc
from concourse.bass_utils import run_bass_kernel_spmd

f32 = mybir.dt.float32
u32 = mybir.dt.uint32
i32 = mybir.dt.int32

N_CORES = 8
B, S, H = 4, 4096, 2048
E = 64
TOP_K = 6
T_FULL = B * S              # 16384 tokens
T_CORE = T_FULL // N_CORES  # 2048 tokens per core
KT = H // 128               # 16 contraction tiles
NTT = T_CORE // 128         # 16 token tiles per core
TB = 512                    # tokens per matmul block (PSUM bank = 512 fp32)
PANEL = 2 * TB              # 1024 tokens per super-panel (one packed psum)
NP = T_CORE // PANEL        # 2 super-panels per core

_CACHE = {}


def _build():
    nc = bacc.Bacc("TRN2", target_bir_lowering=False, debug=False)
    xT = nc.dram_tensor("xT", [H, T_CORE], f32, kind="ExternalInput").ap()
    wT = nc.dram_tensor("wT", [H, E], f32, kind="ExternalInput").ap()
    ident = nc.dram_tensor("ident", [E, E], f32, kind="ExternalInput").ap()
    out_w = nc.dram_tensor("out_w", [128, NTT * TOP_K], f32, kind="ExternalOutput").ap()
    out_i = nc.dram_tensor("out_i", [128, NTT * 8], i32, kind="ExternalOutput").ap()

    with tile.TileContext(nc) as tc:
        with (
            tc.tile_pool(name="persist", bufs=1) as persist,
            tc.tile_pool(name="work", bufs=4) as work,
            tc.tile_pool(name="psum", bufs=2, space="PSUM") as psp,
            tc.tile_pool(name="psumT", bufs=4, space="PSUM") as pspT,
            tc.tile_pool(name="psumW", bufs=1, space="PSUM") as pspW,
        ):
            w_all = persist.tile([128, KT * E], f32, tag="w_all")
            nc.sync.dma_start(
                out=w_all.rearrange("p (a e) -> p a e", a=KT),
                in_=wT.rearrange("(a p) e -> p a e", p=128),
            )
            id_t = persist.tile([E, E], f32, tag="ident")
            nc.sync.dma_start(out=id_t, in_=ident)

            # Warmup matmuls: absorb the w/ident DMA waits on the PE so every
            # real (fused fp32) matmul carries at most one semaphore wait.
            ps_warm = pspW.tile([64, 64], f32, tag="ps_warm")
            nc.tensor.matmul(ps_warm, id_t, id_t, start=True, stop=True)
            nc.tensor.matmul(
                ps_warm, w_all[:, 0:64], w_all[:, 0:64], start=True, stop=True
            )

            stage_w = persist.tile([128, NTT * TOP_K], f32, tag="stage_w")
            stage_i = persist.tile([128, NTT * 8], u32, tag="stage_i")

            for q in range(NP):
                # ---- panel DMA: 16 h-tiles of [128, PANEL] ----
                x_tiles = []
                for a in range(KT):
                    xt = persist.tile([128, PANEL], f32, tag=f"x{q}_{a}")
                    nc.sync.dma_start(
                        out=xt,
                        in_=xT[a * 128 : (a + 1) * 128, q * PANEL : (q + 1) * PANEL],
                    )
                    x_tiles.append(xt)

                # ---- packed accumulation: block half=0 -> partitions 0:64,
                #      half=1 -> partitions 64:128 (concurrent col-groups) ----
                ps = psp.tile([128, TB], f32, tag="ps")
                for a in range(KT):
                    w_tile = w_all[:, a * E : (a + 1) * E]
                    for half in range(2):
                        nc.tensor.matmul(
                            ps[half * 64 : (half + 1) * 64, :],
                            w_tile,
                            x_tiles[a][:, half * TB : (half + 1) * TB],
                            start=(a == 0),
                            stop=(a == KT - 1),
                        )

                # ---- per-block epilogue: copy logits.T to SBUF, transpose,
                #      top-k + softmax per 128-token tile ----
                for half in range(2):
                    ltE = work.tile([64, TB], f32, tag="ltE")
                    nc.scalar.copy(
                        out=ltE, in_=ps[half * 64 : (half + 1) * 64, :]
                    )
                    for tt in range(TB // 128):
                        t = (2 * q + half) * (TB // 128) + tt
                        ps_t = pspT.tile([128, E], f32, tag="ps_t")
                        nc.tensor.transpose(
                            ps_t, ltE[:, tt * 128 : (tt + 1) * 128], id_t
                        )

                        m8 = work.tile([128, 8], f32, tag="m8")
                        nc.vector.max(out=m8, in_=ps_t)
                        nc.vector.max_index(
                            stage_i[:, t * 8 : (t + 1) * 8], m8, ps_t
                        )

                        negm = work.tile([128, 1], f32, tag="negm")
                        nc.scalar.mul(negm, m8[:, 0:1], -1.0)
                        expw = work.tile([128, TOP_K], f32, tag="expw")
                        ssum = work.tile([128, 1], f32, tag="ssum")
                        nc.scalar.activation(
                            out=expw,
                            in_=m8[:, 0:TOP_K],
                            func=mybir.ActivationFunctionType.Exp,
                            bias=negm[:, 0:1],
                            scale=1.0,
                            accum_out=ssum[:, 0:1],
                        )
                        rsum = work.tile([128, 1], f32, tag="rsum")
                        nc.vector.reciprocal(rsum, ssum)
                        nc.vector.tensor_scalar_mul(
                            stage_w[:, t * TOP_K : (t + 1) * TOP_K],
                            expw,
                            rsum[:, 0:1],
                        )

                # ---- per-panel output DMAs ----
                nt_half = NTT // NP  # token tiles per panel
                c0 = q * nt_half
                nc.gpsimd.dma_start(
                    out=out_w[:, c0 * TOP_K : (c0 + nt_half) * TOP_K],
                    in_=stage_w[:, c0 * TOP_K : (c0 + nt_half) * TOP_K],
                )
                nc.gpsimd.dma_start(
                    out=out_i[:, c0 * 8 : (c0 + nt_half) * 8],
                    in_=stage_i[:, c0 * 8 : (c0 + nt_half) * 8].bitcast(i32),
                )

    nc.compile()
    return nc


def _get_nc():
    if "nc" not in _CACHE:
        _CACHE["nc"] = _build()
    return _CACHE["nc"]


def kernel(hidden_states: np.ndarray, weight: np.ndarray, **_run_kwargs):
    x = np.ascontiguousarray(hidden_states, dtype=np.float32).reshape(T_FULL, H)
    w = np.ascontiguousarray(weight, dtype=np.float32)

    wT = np.ascontiguousarray(w.T)  # [H, E]
    ident = np.eye(E, dtype=np.float32)
    in_maps = []
    for c in range(N_CORES):
        shard = x[c * T_CORE : (c + 1) * T_CORE, :]  # [T_CORE, H]
        xT = np.ascontiguousarray(shard.T)  # [H, T_CORE]
        in_maps.append({"xT": xT, "wT": wT, "ident": ident})

    nc = _get_nc()
    res = run_bass_kernel_spmd(
        nc, in_maps, core_ids=list(range(N_CORES)), **_run_kwargs
    )

    idx_parts = []
    w_parts = []
    for c in range(N_CORES):
        r = res.results[c]
        # stage_w [128, NTT*6]: row p, col t*6+k -> token t*128+p, slot k
        # stage_i [128, NTT*8]: row p, col t*8+k -> token t*128+p, slot k (k<6)
        si = r["out_i"].reshape(128, NTT, 8).transpose(1, 0, 2)[:, :, :TOP_K]
        sw = r["out_w"].reshape(128, NTT, TOP_K).transpose(1, 0, 2)
        idx_parts.append(si.reshape(T_CORE, TOP_K).astype(np.int32, copy=False))
        w_parts.append(sw.reshape(T_CORE, TOP_K))

    topk_idx = np.concatenate(idx_parts, axis=0)
    topk_weight = np.concatenate(w_parts, axis=0)
    if "trace" in _run_kwargs:
        return (topk_idx, topk_weight), res
    return topk_idx, topk_weight
